# revision 1
# baseline (speedup 1.0000x reference)
"""Trainium2 Bass kernel for a soft-logic layer (BaseLogicLayer forward).

Computation (reference semantics):
    gw     = softmax(weights, axis=-1)            # (O, 16)
    coeffs = gw @ OP_BASIS                        # (O, 4)
    a      = x[:, selected_inputs[:, 0]]          # (B, O)
    b      = x[:, selected_inputs[:, 1]]          # (B, O)
    out    = c0 + c1*a + c2*b + c3*(a*b)          # (B, O)

Strategy: hybrid sharding across the 8 NeuronCores — 2 batch groups x 4
output groups.  Each core gets its batch half of x pre-transposed
(xT: (IN_DIM, 2048) row-major) so column gathers of x become contiguous-row
gathers (8 KB per index), done on-device with the SWDGE dma_gather
instruction (int16 indices).  The 2x4 split keeps the per-core index count at
8192 because the Q7 descriptor-generation rate (~10 ns/index, serial on the
Pool engine) is the binding constraint at finer batch shards, while HBM
volume (64 MiB gather reads + 32 MiB output writes per core) is invariant.
Each block's idx0 and idx1 lists are fused into a single 512-index
dma_gather (a rows land in the tile's first half, b rows in the second),
alternating between two SWDGE queues per block.

Compute runs with output neurons on partitions so the four per-neuron
coefficients apply as per-partition scalars (ACT scale/bias + DVE
scalar_tensor_tensor, computed in place in one tile); the constant term c0 is
seeded into PSUM by a K=128 matmul against a broadcast identity column, and
PE transpose-accumulates 128x128 blocks on top, yielding the natural
(batch, out) layout.  Transposed blocks are packed 4-per-PSUM-bank, copied to
SBUF by the scalar engine, and stored with 2 KB-contiguous descriptors (two
256-neuron blocks accumulated per store).
"""

import numpy as np

P = 128
B_FULL, IN_DIM, OUT_DIM = 4096, 4096, 16384
N_CORES = 8
BGRP = 2                        # batch groups (shards of x)
OGRP = 4                        # output groups; BGRP*OGRP == N_CORES
BC = B_FULL // BGRP             # 2048 batch rows per core
OD = OUT_DIM // OGRP            # 4096 output neurons per core
BLK = 256                       # output neurons per gather block
NPK = 4                         # transposed 128x128 b-subtiles packed per PSUM bank

_OP_BASIS = np.array([
    [0.,  0.,  0.,  0.],
    [0.,  0.,  0.,  1.],
    [0.,  1.,  0., -1.],
    [0.,  1.,  0.,  0.],
    [0.,  0.,  1., -1.],
    [0.,  0.,  1.,  0.],
    [0.,  1.,  1., -2.],
    [0.,  1.,  1., -1.],
    [1., -1., -1.,  1.],
    [1., -1., -1.,  2.],
    [1.,  0., -1.,  0.],
    [1.,  0., -1.,  1.],
    [1., -1.,  0.,  0.],
    [1., -1.,  0.,  1.],
    [1.,  0.,  0., -1.],
    [1.,  0.,  0.,  0.],
], dtype=np.float32)


def _build_nc(bc=BC, in_dim=IN_DIM, out_dim=OD, blk=BLK, reps=1, bench_sink=False, parts='all'):
    import concourse.bacc as bacc
    import concourse.mybir as mybir
    import concourse.tile as tile
    from concourse.masks import make_identity
    from concourse.library_config import mlp

    f32 = mybir.dt.float32
    i16 = mybir.dt.int16
    AF = mybir.ActivationFunctionType
    ALU = mybir.AluOpType
    AX = mybir.AxisListType

    nblk = out_dim // blk
    chunks = blk // P
    nbt = bc // P                 # transposed 128-row batch sub-tiles
    npk = min(NPK, nbt)           # b-subtiles packed per PSUM tile
    npsg = nbt // npk             # PSUM tiles per chunk
    ncg = out_dim // P            # total 128-output chunks (coeff columns)
    ncg_p = min(ncg, P)
    idx_cols = blk // 16
    psum_bufs = max(2, 8 // max(1, npsg))
    otw = 2 if nblk % 2 == 0 else 1      # blocks accumulated per output store

    nc = bacc.Bacc("TRN2", target_bir_lowering=False, debug=False,
                   num_swdge_queues=2)
    xt = nc.dram_tensor("xt", [in_dim, bc], f32, kind="ExternalInput")
    wq = nc.dram_tensor("wq", [P, ncg * 16], f32, kind="ExternalInput")
    basis = nc.dram_tensor("basis", [P, 64], f32, kind="ExternalInput")
    idxd = nc.dram_tensor("idx", [P, 2 * nblk * idx_cols], i16, kind="ExternalInput")
    if bench_sink:
        out = nc.dram_tensor("sink", [bc, out_dim], f32, kind="Internal")
        tiny = nc.dram_tensor("out", [P, 16], f32, kind="ExternalOutput")
    else:
        out = nc.dram_tensor("out", [bc, out_dim], f32, kind="ExternalOutput")
        tiny = None

    with tile.TileContext(nc) as tc:
        with (
            tc.tile_pool(name="const", bufs=1) as constp,
            tc.tile_pool(name="gather", bufs=2) as gp,
            tc.tile_pool(name="chunk", bufs=4) as cp,
            tc.tile_pool(name="ot", bufs=2) as otp,
            tc.tile_pool(name="psum", bufs=psum_bufs, space="PSUM") as pp,
        ):
            nc.gpsimd.load_library(mlp)

            ident = constp.tile([P, P], f32)
            make_identity(nc, ident[:])

            idxt = constp.tile([P, 2 * nblk * idx_cols], i16)
            nc.sync.dma_start(idxt[:], idxd[:, :])

            # --- coefficients: softmax(weights) @ OP_BASIS, all on-chip ---
            wt = constp.tile([P, ncg * 16], f32)
            nc.sync.dma_start(wt[:], wq[:, :])
            bt = constp.tile([P, 64], f32)
            nc.sync.dma_start(bt[:], basis[:, :])

            ew = constp.tile([P, ncg * 16], f32)
            # |weights| ~ 0.1*N(0,1): exp without max-subtraction is safe
            nc.scalar.activation(ew[:], wt[:], AF.Exp)
            ew3 = ew[:].rearrange("p (c k) -> p c k", k=16)
            ssum = constp.tile([P, ncg], f32)
            nc.vector.tensor_reduce(ssum[:], ew3, axis=AX.X, op=ALU.add)
            rcp = constp.tile([P, ncg], f32)
            nc.vector.reciprocal(rcp[:], ssum[:])

            C = []
            scratch = constp.tile([P, ncg * 16], f32)
            s3 = scratch[:].rearrange("p (c k) -> p c k", k=16)
            acc = constp.tile([P, ncg], f32)
            for j in range(4):
                bj = bt[:, j * 16:(j + 1) * 16].unsqueeze(1).broadcast_to(
                    [P, ncg, 16])
                nc.vector.tensor_tensor(s3, ew3, bj, op=ALU.mult)
                nc.vector.tensor_reduce(acc[:], s3, axis=AX.X, op=ALU.add)
                cj = constp.tile([P, ncg], f32, tag=f"c{j}", name=f"c{j}")
                nc.vector.tensor_tensor(cj[:], acc[:], rcp[:], op=ALU.mult)
                C.append(cj)

            # c0 transposed to [cg, p] layout: the constant term is seeded
            # into PSUM as ident[:, cg].bcast.T @ c0t (= row cg broadcast
            # over all partitions) before the PE transpose accumulates
            c0t_ps = pp.tile([P, npk * P], f32, tag="ps0", name="c0t_ps")
            nc.tensor.transpose(c0t_ps[:ncg_p, :P], C[0][:, :ncg_p], ident[:])
            c0t = constp.tile([P, P], f32)
            nc.gpsimd.memset(c0t[:], 0.0)
            nc.vector.tensor_copy(c0t[:ncg_p, :], c0t_ps[:ncg_p, :P])

            # --- main loop: gather, combine, transpose, store ---
            otb_holder = [None]

            def _main_body():
              for bi in range(nblk):
                  gt = gp.tile([P, 2 * chunks, bc], f32, tag="g", name="gt")
                  iab = idxt[:, (2 * bi) * idx_cols:(2 * bi + 2) * idx_cols]
                  if parts in ('all', 'gather'):
                      nc.gpsimd.dma_gather(gt[:], xt[:, :], iab, 2 * blk,
                                           2 * blk, bc, queue_num=bi % 2)

                  if parts == 'gather':
                      continue

                  if bi % otw == 0:
                      otb_holder[0] = otp.tile(
                          [P, nbt, otw * blk], f32, tag="otb", name="otb")
                  otb = otb_holder[0]
                  obase = (bi % otw) * blk
                  for c in range(chunks):
                      cg = bi * chunks + c
                      a = gt[:, c, :]
                      b = gt[:, chunks + c, :]
                      r = cp.tile([P, bc], f32, tag="u")
                      nc.scalar.activation(
                          r[:], a, AF.Identity,
                          bias=C[2][:, cg:cg + 1], scale=C[3][:, cg:cg + 1])
                      nc.vector.tensor_tensor(r[:], r[:], b, op=ALU.mult)
                      nc.vector.scalar_tensor_tensor(
                          r[:], a, C[1][:, cg:cg + 1], r[:],
                          op0=ALU.mult, op1=ALU.add)
                      sel = ident[:, cg % P:cg % P + 1].to_broadcast([P, P])
                      c0rhs = c0t[:, :].unsqueeze(1).broadcast_to([P, npk, P])
                      for j in range(npsg):
                          psj = pp.tile([P, npk * P], f32, tag=f"ps{j % 4}",
                                        name=f"ps{j % 4}")
                          nc.tensor.matmul(
                              out=psj[:], lhsT=sel, rhs=c0rhs,
                              start=True, stop=False, skip_group_check=True)
                          for k in range(npk):
                              s = j * npk + k
                              sl = psj[:, k * P:(k + 1) * P]
                              nc.tensor.matmul(
                                  out=sl, lhsT=r[:, s * P:(s + 1) * P],
                                  rhs=ident[:], is_transpose=True,
                                  start=False, stop=True, skip_group_check=True)
                          dst = otb[:, j * npk:(j + 1) * npk,
                                    obase + c * P:obase + (c + 1) * P]
                          nc.scalar.copy(dst, psj[:].rearrange(
                              "p (k o) -> p k o", k=npk))
                  if bi % otw == otw - 1:
                      o0 = (bi - otw + 1) * blk
                      nc.sync.dma_start(
                          out[:, o0:o0 + otw * blk].rearrange(
                              "(s p) o -> p s o", p=P),
                          otb[:])

            if reps == 1:
                _main_body()
            else:
                with tc.For_i(0, reps, 1):
                    _main_body()
            if tiny is not None:
                nc.sync.dma_start(tiny[:, :], C[0][:, 0:16])
    nc.compile()
    return nc


def _wrap_idx(seg):
    """idx list (n,) -> (128, n//16) int16 in the dma_gather wrapped layout:
    position j lives at [j % 16, j // 16], replicated across partition
    groups of 16."""
    n = seg.shape[0]
    w = seg.reshape(n // 16, 16).T.astype(np.int16)     # (16, n//16)
    return np.tile(w, (8, 1))                           # (128, n//16)


def _prep_inputs(x, weights, selected_inputs):
    x = np.asarray(x, dtype=np.float32)
    w = np.asarray(weights, dtype=np.float32)
    si = np.asarray(selected_inputs).astype(np.int64)

    # x transposed per batch group (shared by the OGRP cores of each group)
    xts = [np.ascontiguousarray(x[g * BC:(g + 1) * BC, :].T) for g in range(BGRP)]

    basis = np.ascontiguousarray(
        np.tile(_OP_BASIS.T.reshape(1, 64), (P, 1)).astype(np.float32))

    # per output group: rearranged weights + wrapped idx
    ncg = OD // P
    nblk = OD // BLK
    wqs, idxs = [], []
    for og in range(OGRP):
        wsh = w[og * OD:(og + 1) * OD]
        wqs.append(np.ascontiguousarray(
            wsh.reshape(ncg, P, 16).transpose(1, 0, 2).reshape(P, ncg * 16)))
        sish = si[og * OD:(og + 1) * OD]
        parts = []
        for bi in range(nblk):
            seg = np.concatenate(
                [sish[bi * BLK:(bi + 1) * BLK, 0],
                 sish[bi * BLK:(bi + 1) * BLK, 1]])
            parts.append(_wrap_idx(seg))
        idxs.append(np.ascontiguousarray(np.concatenate(parts, axis=1)))

    in_maps = []
    for c in range(N_CORES):
        bg, og = divmod(c, OGRP)
        in_maps.append(
            {"xt": xts[bg], "wq": wqs[og], "basis": basis, "idx": idxs[og]})
    return in_maps


_last_results = None


def kernel(x, weights, selected_inputs):
    global _last_results
    from concourse import bass_utils

    in_maps = _prep_inputs(x, weights, selected_inputs)
    nc = _build_nc()
    res = bass_utils.run_bass_kernel_spmd(
        nc, in_maps, core_ids=list(range(N_CORES)))
    _last_results = res
    out = np.empty((B_FULL, OUT_DIM), dtype=np.float32)
    for c in range(N_CORES):
        bg, og = divmod(c, OGRP)
        out[bg * BC:(bg + 1) * BC, og * OD:(og + 1) * OD] = res.results[c]["out"]
    return out



# revision 2
# speedup vs baseline: 30.1668x; 30.1668x over previous
"""Trainium2 Bass kernel for a soft-logic layer (BaseLogicLayer forward).

Computation (reference semantics):
    gw     = softmax(weights, axis=-1)            # (O, 16)
    coeffs = gw @ OP_BASIS                        # (O, 4)
    a      = x[:, selected_inputs[:, 0]]          # (B, O)
    b      = x[:, selected_inputs[:, 1]]          # (B, O)
    out    = c0 + c1*a + c2*b + c3*(a*b)          # (B, O)

Strategy: hybrid sharding across the 8 NeuronCores — 2 batch groups x 4
output groups.  Each core gets its batch half of x pre-transposed AND
quantized to bf16 on the host (xT: (IN_DIM, 2048) row-major) so column
gathers of x become contiguous-row gathers (4 KB per index), done
on-device with the SWDGE dma_gather instruction (int16 indices).  The
kernel is DMA-bandwidth-bound: bf16 halves both dominant HBM streams
(gather reads 32 MiB/core, output writes 16 MiB/core as bf16 that the
host upconverts to f32; tolerance is 2e-2 rel, bf16 costs ~3e-3).

Compute runs with output neurons on partitions so the four per-neuron
coefficients apply as per-partition scalars (ACT scale/bias + DVE
scalar_tensor_tensor, f32 intermediates); the constant term c0 is seeded
into PSUM by a matmul against a broadcast identity column, and PE
transpose-accumulates 128x128 blocks on top, yielding the natural
(batch, out) layout.  Transposed blocks are packed 4-per-PSUM-bank,
copied to SBUF (converting f32->bf16), and stored with 2 KB-contiguous
descriptors (four 256-neuron blocks accumulated per store).
"""

import numpy as np

P = 128
B_FULL, IN_DIM, OUT_DIM = 4096, 4096, 16384
N_CORES = 8
BGRP = 2                        # batch groups (shards of x)
OGRP = 4                        # output groups; BGRP*OGRP == N_CORES
BC = B_FULL // BGRP             # 2048 batch rows per core
OD = OUT_DIM // OGRP            # 4096 output neurons per core
BLK = 256                       # output neurons per gather block
NPK = 4                         # transposed 128x128 b-subtiles packed per PSUM bank
OTW = 4                         # gather blocks accumulated per output store

_OP_BASIS = np.array([
    [0.,  0.,  0.,  0.],
    [0.,  0.,  0.,  1.],
    [0.,  1.,  0., -1.],
    [0.,  1.,  0.,  0.],
    [0.,  0.,  1., -1.],
    [0.,  0.,  1.,  0.],
    [0.,  1.,  1., -2.],
    [0.,  1.,  1., -1.],
    [1., -1., -1.,  1.],
    [1., -1., -1.,  2.],
    [1.,  0., -1.,  0.],
    [1.,  0., -1.,  1.],
    [1., -1.,  0.,  0.],
    [1., -1.,  0.,  1.],
    [1.,  0.,  0., -1.],
    [1.,  0.,  0.,  0.],
], dtype=np.float32)


def _build_nc(bc=BC, in_dim=IN_DIM, out_dim=OD, blk=BLK, reps=1, bench_sink=False,
              parts='all'):
    import concourse.bacc as bacc
    import concourse.mybir as mybir
    import concourse.tile as tile
    from concourse.masks import make_identity
    from concourse.library_config import mlp

    f32 = mybir.dt.float32
    bf16 = mybir.dt.bfloat16
    i16 = mybir.dt.int16
    AF = mybir.ActivationFunctionType
    ALU = mybir.AluOpType
    AX = mybir.AxisListType

    nblk = out_dim // blk
    chunks = blk // P
    nbt = bc // P                 # transposed 128-row batch sub-tiles
    npk = min(NPK, nbt)           # b-subtiles packed per PSUM tile
    npsg = nbt // npk             # PSUM tiles per chunk
    ncg = out_dim // P            # total 128-output chunks (coeff columns)
    ncg_p = min(ncg, P)
    idx_cols = blk // 16
    psum_bufs = max(2, 8 // max(1, npsg))
    otw = OTW
    while nblk % otw:
        otw //= 2

    nc = bacc.Bacc("TRN2", target_bir_lowering=False, debug=False,
                   num_swdge_queues=2)
    # bench mode: xt stays device-resident garbage (DMA/compute time is
    # value-independent) so per-call upload is tiny and the rep-slope is clean
    xt_kind = "Internal" if bench_sink else "ExternalInput"
    xt = nc.dram_tensor("xt", [in_dim, bc], bf16, kind=xt_kind)
    wq = nc.dram_tensor("wq", [P, ncg * 16], f32, kind="ExternalInput")
    basis = nc.dram_tensor("basis", [P, 64], f32, kind="ExternalInput")
    idxd = nc.dram_tensor("idx", [P, 2 * nblk * idx_cols], i16, kind="ExternalInput")
    if bench_sink:
        out = nc.dram_tensor("sink", [bc, out_dim], bf16, kind="Internal")
        tiny = nc.dram_tensor("out", [P, 16], f32, kind="ExternalOutput")
    else:
        out = nc.dram_tensor("out", [bc, out_dim], bf16, kind="ExternalOutput")
        tiny = None

    with tile.TileContext(nc) as tc:
        with (
            tc.tile_pool(name="const", bufs=1) as constp,
            tc.tile_pool(name="gather", bufs=2) as gp,
            tc.tile_pool(name="chunk", bufs=4) as cp,
            tc.tile_pool(name="ot", bufs=2) as otp,
            tc.tile_pool(name="psum", bufs=psum_bufs, space="PSUM") as pp,
        ):
            nc.gpsimd.load_library(mlp)

            ident = constp.tile([P, P], f32)
            make_identity(nc, ident[:])

            idxt = constp.tile([P, 2 * nblk * idx_cols], i16)
            nc.sync.dma_start(idxt[:], idxd[:, :])

            # --- coefficients: softmax(weights) @ OP_BASIS, all on-chip ---
            wt = constp.tile([P, ncg * 16], f32)
            nc.sync.dma_start(wt[:], wq[:, :])
            bt = constp.tile([P, 64], f32)
            nc.sync.dma_start(bt[:], basis[:, :])

            ew = constp.tile([P, ncg * 16], f32)
            # |weights| ~ 0.1*N(0,1): exp without max-subtraction is safe
            nc.scalar.activation(ew[:], wt[:], AF.Exp)
            ew3 = ew[:].rearrange("p (c k) -> p c k", k=16)
            ssum = constp.tile([P, ncg], f32)
            nc.vector.tensor_reduce(ssum[:], ew3, axis=AX.X, op=ALU.add)
            rcp = constp.tile([P, ncg], f32)
            nc.vector.reciprocal(rcp[:], ssum[:])

            C = []
            scratch = constp.tile([P, ncg * 16], f32)
            s3 = scratch[:].rearrange("p (c k) -> p c k", k=16)
            acc = constp.tile([P, ncg], f32)
            for j in range(4):
                bj = bt[:, j * 16:(j + 1) * 16].unsqueeze(1).broadcast_to(
                    [P, ncg, 16])
                nc.vector.tensor_tensor(s3, ew3, bj, op=ALU.mult)
                nc.vector.tensor_reduce(acc[:], s3, axis=AX.X, op=ALU.add)
                cj = constp.tile([P, ncg], f32, tag=f"c{j}", name=f"c{j}")
                nc.vector.tensor_tensor(cj[:], acc[:], rcp[:], op=ALU.mult)
                C.append(cj)

            # c0 transposed to [cg, p] layout: the constant term is seeded
            # into PSUM as ident[:, cg].bcast.T @ c0t (= row cg broadcast
            # over all partitions) before the PE transpose accumulates
            c0t_ps = pp.tile([P, npk * P], f32, tag="ps0", name="c0t_ps")
            nc.tensor.transpose(c0t_ps[:ncg_p, :P], C[0][:, :ncg_p], ident[:])
            c0t = constp.tile([P, P], f32)
            nc.gpsimd.memset(c0t[:], 0.0)
            nc.vector.tensor_copy(c0t[:ncg_p, :], c0t_ps[:ncg_p, :P])

            # --- main loop: gather, combine, transpose, store ---
            otb_holder = [None]

            def _main_body():
              for bi in range(nblk):
                  gt = gp.tile([P, 2 * chunks, bc], bf16, tag="g", name="gt")
                  iab = idxt[:, (2 * bi) * idx_cols:(2 * bi + 2) * idx_cols]
                  if parts in ('all', 'gather'):
                      nc.gpsimd.dma_gather(gt[:], xt[:, :], iab, 2 * blk,
                                           2 * blk, bc, queue_num=bi % 2)

                  if parts == 'gather':
                      continue

                  if bi % otw == 0:
                      otb_holder[0] = otp.tile(
                          [P, nbt, otw * blk], bf16, tag="otb", name="otb")
                  otb = otb_holder[0]
                  obase = (bi % otw) * blk
                  for c in range(chunks):
                      cg = bi * chunks + c
                      a = gt[:, c, :]
                      b = gt[:, chunks + c, :]
                      r = cp.tile([P, bc], f32, tag="u")
                      nc.scalar.activation(
                          r[:], a, AF.Identity,
                          bias=C[2][:, cg:cg + 1], scale=C[3][:, cg:cg + 1])
                      nc.vector.tensor_tensor(r[:], r[:], b, op=ALU.mult)
                      nc.vector.scalar_tensor_tensor(
                          r[:], a, C[1][:, cg:cg + 1], r[:],
                          op0=ALU.mult, op1=ALU.add)
                      sel = ident[:, cg % P:cg % P + 1].to_broadcast([P, P])
                      c0rhs = c0t[:, :].unsqueeze(1).broadcast_to([P, npk, P])
                      for j in range(npsg):
                          psj = pp.tile([P, npk * P], f32, tag=f"ps{j % 4}",
                                        name=f"ps{j % 4}")
                          nc.tensor.matmul(
                              out=psj[:], lhsT=sel, rhs=c0rhs,
                              start=True, stop=False, skip_group_check=True)
                          for k in range(npk):
                              s = j * npk + k
                              sl = psj[:, k * P:(k + 1) * P]
                              nc.tensor.matmul(
                                  out=sl, lhsT=r[:, s * P:(s + 1) * P],
                                  rhs=ident[:], is_transpose=True,
                                  start=False, stop=True, skip_group_check=True)
                          dst = otb[:, j * npk:(j + 1) * npk,
                                    obase + c * P:obase + (c + 1) * P]
                          nc.scalar.copy(dst, psj[:].rearrange(
                              "p (k o) -> p k o", k=npk))
                  if bi % otw == otw - 1:
                      o0 = (bi - otw + 1) * blk
                      nc.sync.dma_start(
                          out[:, o0:o0 + otw * blk].rearrange(
                              "(s p) o -> p s o", p=P),
                          otb[:])

            if reps == 1:
                _main_body()
            else:
                with tc.For_i(0, reps, 1):
                    _main_body()
            if tiny is not None:
                nc.sync.dma_start(tiny[:, :], C[0][:, 0:16])
    nc.compile()
    return nc


def _wrap_idx(seg):
    """idx list (n,) -> (128, n//16) int16 in the dma_gather wrapped layout:
    position j lives at [j % 16, j // 16], replicated across partition
    groups of 16."""
    n = seg.shape[0]
    w = seg.reshape(n // 16, 16).T.astype(np.int16)     # (16, n//16)
    return np.tile(w, (8, 1))                           # (128, n//16)


def _prep_inputs(x, weights, selected_inputs):
    import ml_dtypes

    x = np.asarray(x, dtype=np.float32)
    w = np.asarray(weights, dtype=np.float32)
    si = np.asarray(selected_inputs).astype(np.int64)

    # x transposed per batch group (shared by the OGRP cores of each group),
    # quantized to bf16 on the host
    xts = [np.ascontiguousarray(x[g * BC:(g + 1) * BC, :].T.astype(
        ml_dtypes.bfloat16)) for g in range(BGRP)]

    basis = np.ascontiguousarray(
        np.tile(_OP_BASIS.T.reshape(1, 64), (P, 1)).astype(np.float32))

    # per output group: rearranged weights + wrapped idx
    ncg = OD // P
    nblk = OD // BLK
    wqs, idxs = [], []
    for og in range(OGRP):
        wsh = w[og * OD:(og + 1) * OD]
        wqs.append(np.ascontiguousarray(
            wsh.reshape(ncg, P, 16).transpose(1, 0, 2).reshape(P, ncg * 16)))
        sish = si[og * OD:(og + 1) * OD]
        parts = []
        for bi in range(nblk):
            seg = np.concatenate(
                [sish[bi * BLK:(bi + 1) * BLK, 0],
                 sish[bi * BLK:(bi + 1) * BLK, 1]])
            parts.append(_wrap_idx(seg))
        idxs.append(np.ascontiguousarray(np.concatenate(parts, axis=1)))

    in_maps = []
    for c in range(N_CORES):
        bg, og = divmod(c, OGRP)
        in_maps.append(
            {"xt": xts[bg], "wq": wqs[og], "basis": basis, "idx": idxs[og]})
    return in_maps


_last_results = None


def kernel(x, weights, selected_inputs):
    global _last_results
    from concourse import bass_utils

    in_maps = _prep_inputs(x, weights, selected_inputs)
    nc = _build_nc()
    res = bass_utils.run_bass_kernel_spmd(
        nc, in_maps, core_ids=list(range(N_CORES)))
    _last_results = res
    out = np.empty((B_FULL, OUT_DIM), dtype=np.float32)
    for c in range(N_CORES):
        bg, og = divmod(c, OGRP)
        out[bg * BC:(bg + 1) * BC, og * OD:(og + 1) * OD] = (
            np.asarray(res.results[c]["out"]).astype(np.float32))
    return out


# revision 3
# speedup vs baseline: 208.4559x; 6.9101x over previous
"""Trainium2 Bass kernel for a soft-logic layer (BaseLogicLayer forward).

Computation (reference semantics):
    gw     = softmax(weights, axis=-1)            # (O, 16)
    coeffs = gw @ OP_BASIS                        # (O, 4)
    a      = x[:, selected_inputs[:, 0]]          # (B, O)
    b      = x[:, selected_inputs[:, 1]]          # (B, O)
    out    = c0 + c1*a + c2*b + c3*(a*b)          # (B, O)

Strategy: hybrid sharding across the 8 NeuronCores — 2 batch groups x 4
output groups.  Each core gets its batch half of x pre-transposed AND
quantized to bf16 on the host (xT: (IN_DIM, 2048) row-major) so column
gathers of x become contiguous-row gathers (4 KB per index), done
on-device with the SWDGE dma_gather instruction (int16 indices).  The
kernel is DMA-bandwidth-bound: bf16 halves both dominant HBM streams
(gather reads 32 MiB/core, output writes 16 MiB/core as bf16 that the
host upconverts to f32; tolerance is 2e-2 rel, bf16 costs ~3e-3).

Compute runs with output neurons on partitions so the four per-neuron
coefficients apply as per-partition scalars (ACT scale/bias + DVE
scalar_tensor_tensor, f32 intermediates); the constant term c0 is seeded
into PSUM by a matmul against a broadcast identity column, and PE
transpose-accumulates 128x128 blocks on top, yielding the natural
(batch, out) layout.  Transposed blocks are packed 4-per-PSUM-bank,
copied to SBUF (converting f32->bf16), and stored with 2 KB-contiguous
descriptors (four 256-neuron blocks accumulated per store).
"""

import numpy as np

P = 128
B_FULL, IN_DIM, OUT_DIM = 4096, 4096, 16384
N_CORES = 8
BGRP = 2                        # batch groups (shards of x)
OGRP = 4                        # output groups; BGRP*OGRP == N_CORES
BC = B_FULL // BGRP             # 2048 batch rows per core
OD = OUT_DIM // OGRP            # 4096 output neurons per core
BLK = 256                       # output neurons per gather block
NPK = 4                         # transposed 128x128 b-subtiles packed per PSUM bank
OTW = 4                         # gather blocks accumulated per output store

_OP_BASIS = np.array([
    [0.,  0.,  0.,  0.],
    [0.,  0.,  0.,  1.],
    [0.,  1.,  0., -1.],
    [0.,  1.,  0.,  0.],
    [0.,  0.,  1., -1.],
    [0.,  0.,  1.,  0.],
    [0.,  1.,  1., -2.],
    [0.,  1.,  1., -1.],
    [1., -1., -1.,  1.],
    [1., -1., -1.,  2.],
    [1.,  0., -1.,  0.],
    [1.,  0., -1.,  1.],
    [1., -1.,  0.,  0.],
    [1., -1.,  0.,  1.],
    [1.,  0.,  0., -1.],
    [1.,  0.,  0.,  0.],
], dtype=np.float32)


def _build_nc(bc=BC, in_dim=IN_DIM, out_dim=OD, blk=BLK, reps=1, bench_sink=False,
              parts='all'):
    import concourse.bacc as bacc
    import concourse.mybir as mybir
    import concourse.tile as tile
    from concourse.masks import make_identity
    from concourse.library_config import mlp

    f32 = mybir.dt.float32
    bf16 = mybir.dt.bfloat16
    i16 = mybir.dt.int16
    AF = mybir.ActivationFunctionType
    ALU = mybir.AluOpType
    AX = mybir.AxisListType

    nblk = out_dim // blk
    chunks = blk // P
    nbt = bc // P                 # transposed 128-row batch sub-tiles
    npk = min(NPK, nbt)           # b-subtiles packed per PSUM tile
    npsg = nbt // npk             # PSUM tiles per chunk
    ncg = out_dim // P            # total 128-output chunks (coeff columns)
    ncg_p = min(ncg, P)
    idx_cols = blk // 16
    psum_bufs = max(2, 8 // max(1, npsg))
    otw = OTW
    while nblk % otw:
        otw //= 2

    nc = bacc.Bacc("TRN2", target_bir_lowering=False, debug=False,
                   num_swdge_queues=2)
    # bench mode: xt stays device-resident garbage (DMA/compute time is
    # value-independent) so per-call upload is tiny and the rep-slope is clean
    xt_kind = "Internal" if bench_sink else "ExternalInput"
    xt = nc.dram_tensor("xt", [in_dim, bc], bf16, kind=xt_kind)
    wq = nc.dram_tensor("wq", [P, ncg * 16], f32, kind="ExternalInput")
    basis = nc.dram_tensor("basis", [P, 64], f32, kind="ExternalInput")
    idxd = nc.dram_tensor("idx", [P, 2 * nblk * idx_cols], i16, kind="ExternalInput")
    if bench_sink:
        out = nc.dram_tensor("sink", [bc, out_dim], bf16, kind="Internal")
        tiny = nc.dram_tensor("out", [P, 16], f32, kind="ExternalOutput")
    else:
        out = nc.dram_tensor("out", [bc, out_dim], bf16, kind="ExternalOutput")
        tiny = None

    with tile.TileContext(nc) as tc:
        with (
            tc.tile_pool(name="const", bufs=1) as constp,
            tc.tile_pool(name="gather", bufs=2) as gp,
            tc.tile_pool(name="chunk", bufs=4) as cp,
            tc.tile_pool(name="ot", bufs=2) as otp,
            tc.tile_pool(name="psum", bufs=psum_bufs, space="PSUM") as pp,
        ):
            nc.gpsimd.load_library(mlp)

            ident = constp.tile([P, P], f32)
            make_identity(nc, ident[:])

            idxt = constp.tile([P, 2 * nblk * idx_cols], i16)
            nc.sync.dma_start(idxt[:], idxd[:, :])

            # --- coefficients: softmax(weights) @ OP_BASIS, all on-chip ---
            wt = constp.tile([P, ncg * 16], f32)
            nc.sync.dma_start(wt[:], wq[:, :])
            bt = constp.tile([P, 64], f32)
            nc.sync.dma_start(bt[:], basis[:, :])

            ew = constp.tile([P, ncg * 16], f32)
            # |weights| ~ 0.1*N(0,1): exp without max-subtraction is safe
            nc.scalar.activation(ew[:], wt[:], AF.Exp)
            ew3 = ew[:].rearrange("p (c k) -> p c k", k=16)
            ssum = constp.tile([P, ncg], f32)
            nc.vector.tensor_reduce(ssum[:], ew3, axis=AX.X, op=ALU.add)
            rcp = constp.tile([P, ncg], f32)
            nc.vector.reciprocal(rcp[:], ssum[:])

            C = []
            scratch = constp.tile([P, ncg * 16], f32)
            s3 = scratch[:].rearrange("p (c k) -> p c k", k=16)
            acc = constp.tile([P, ncg], f32)
            for j in range(4):
                bj = bt[:, j * 16:(j + 1) * 16].unsqueeze(1).broadcast_to(
                    [P, ncg, 16])
                nc.vector.tensor_tensor(s3, ew3, bj, op=ALU.mult)
                nc.vector.tensor_reduce(acc[:], s3, axis=AX.X, op=ALU.add)
                cj = constp.tile([P, ncg], f32, tag=f"c{j}", name=f"c{j}")
                nc.vector.tensor_tensor(cj[:], acc[:], rcp[:], op=ALU.mult)
                C.append(cj)

            # c0 transposed to [cg, p] layout: the constant term is seeded
            # into PSUM as ident[:, cg].bcast.T @ c0t (= row cg broadcast
            # over all partitions) before the PE transpose accumulates
            c0t_ps = pp.tile([P, npk * P], f32, tag="ps0", name="c0t_ps")
            nc.tensor.transpose(c0t_ps[:ncg_p, :P], C[0][:, :ncg_p], ident[:])
            c0t = constp.tile([P, P], f32)
            nc.gpsimd.memset(c0t[:], 0.0)
            nc.vector.tensor_copy(c0t[:ncg_p, :], c0t_ps[:ncg_p, :P])

            # --- main loop: gather, combine, transpose, store ---
            otb_holder = [None]

            def _main_body():
              for bi in range(nblk):
                  gt = gp.tile([P, 2 * chunks, bc], bf16, tag="g", name="gt")
                  iab = idxt[:, (2 * bi) * idx_cols:(2 * bi + 2) * idx_cols]
                  if parts in ('all', 'gather'):
                      nc.gpsimd.dma_gather(gt[:], xt[:, :], iab, 2 * blk,
                                           2 * blk, bc, queue_num=bi % 2)

                  if parts == 'gather':
                      continue
                  do_store = parts in ('all', 'nogather', 'store')
                  if parts == 'store':
                      if bi % otw == 0:
                          otb_holder[0] = otp.tile(
                              [P, nbt, otw * blk], bf16, tag="otb", name="otb")
                          nc.vector.memset(otb_holder[0][:, 0, 0:1], 0.0)
                      if bi % otw == otw - 1:
                          o0 = (bi - otw + 1) * blk
                          nc.sync.dma_start(
                              out[:, o0:o0 + otw * blk].rearrange(
                                  "(s p) o -> p s o", p=P),
                              otb_holder[0][:])
                      continue

                  if bi % otw == 0:
                      otb_holder[0] = otp.tile(
                          [P, nbt, otw * blk], bf16, tag="otb", name="otb")
                  otb = otb_holder[0]
                  obase = (bi % otw) * blk
                  for c in range(chunks):
                      cg = bi * chunks + c
                      a = gt[:, c, :]
                      b = gt[:, chunks + c, :]
                      r = cp.tile([P, bc], f32, tag="u")
                      nc.scalar.activation(
                          r[:], a, AF.Identity,
                          bias=C[2][:, cg:cg + 1], scale=C[3][:, cg:cg + 1])
                      nc.vector.tensor_tensor(r[:], r[:], b, op=ALU.mult)
                      nc.vector.scalar_tensor_tensor(
                          r[:], a, C[1][:, cg:cg + 1], r[:],
                          op0=ALU.mult, op1=ALU.add)
                      sel = ident[:, cg % P:cg % P + 1].to_broadcast([P, P])
                      c0rhs = c0t[:, :].unsqueeze(1).broadcast_to([P, npk, P])
                      for j in range(npsg):
                          psj = pp.tile([P, npk * P], f32, tag=f"ps{j % 4}",
                                        name=f"ps{j % 4}")
                          nc.tensor.matmul(
                              out=psj[:], lhsT=sel, rhs=c0rhs,
                              start=True, stop=False, skip_group_check=True)
                          for k in range(npk):
                              s = j * npk + k
                              sl = psj[:, k * P:(k + 1) * P]
                              nc.tensor.matmul(
                                  out=sl, lhsT=r[:, s * P:(s + 1) * P],
                                  rhs=ident[:], is_transpose=True,
                                  start=False, stop=True, skip_group_check=True)
                          dst = otb[:, j * npk:(j + 1) * npk,
                                    obase + c * P:obase + (c + 1) * P]
                          nc.scalar.copy(dst, psj[:].rearrange(
                              "p (k o) -> p k o", k=npk))
                  if bi % otw == otw - 1:
                      o0 = (bi - otw + 1) * blk
                      nc.sync.dma_start(
                          out[:, o0:o0 + otw * blk].rearrange(
                              "(s p) o -> p s o", p=P),
                          otb[:])

            if reps == 1:
                _main_body()
            else:
                with tc.For_i(0, reps, 1):
                    _main_body()
            if tiny is not None:
                nc.sync.dma_start(tiny[:, :], C[0][:, 0:16])
    nc.compile()
    return nc


def _wrap_idx(seg):
    """idx list (n,) -> (128, n//16) int16 in the dma_gather wrapped layout:
    position j lives at [j % 16, j // 16], replicated across partition
    groups of 16."""
    n = seg.shape[0]
    w = seg.reshape(n // 16, 16).T.astype(np.int16)     # (16, n//16)
    return np.tile(w, (8, 1))                           # (128, n//16)


def _prep_inputs(x, weights, selected_inputs):
    import ml_dtypes

    x = np.asarray(x, dtype=np.float32)
    w = np.asarray(weights, dtype=np.float32)
    si = np.asarray(selected_inputs).astype(np.int64)

    # x transposed per batch group (shared by the OGRP cores of each group),
    # quantized to bf16 on the host
    xts = [np.ascontiguousarray(x[g * BC:(g + 1) * BC, :].T.astype(
        ml_dtypes.bfloat16)) for g in range(BGRP)]

    basis = np.ascontiguousarray(
        np.tile(_OP_BASIS.T.reshape(1, 64), (P, 1)).astype(np.float32))

    # per output group: rearranged weights + wrapped idx
    ncg = OD // P
    nblk = OD // BLK
    wqs, idxs = [], []
    for og in range(OGRP):
        wsh = w[og * OD:(og + 1) * OD]
        wqs.append(np.ascontiguousarray(
            wsh.reshape(ncg, P, 16).transpose(1, 0, 2).reshape(P, ncg * 16)))
        sish = si[og * OD:(og + 1) * OD]
        parts = []
        for bi in range(nblk):
            seg = np.concatenate(
                [sish[bi * BLK:(bi + 1) * BLK, 0],
                 sish[bi * BLK:(bi + 1) * BLK, 1]])
            parts.append(_wrap_idx(seg))
        idxs.append(np.ascontiguousarray(np.concatenate(parts, axis=1)))

    in_maps = []
    for c in range(N_CORES):
        bg, og = divmod(c, OGRP)
        in_maps.append(
            {"xt": xts[bg], "wq": wqs[og], "basis": basis, "idx": idxs[og]})
    return in_maps


_last_results = None


def kernel(x, weights, selected_inputs):
    global _last_results
    from concourse import bass_utils

    in_maps = _prep_inputs(x, weights, selected_inputs)
    nc = _build_nc()
    res = bass_utils.run_bass_kernel_spmd(
        nc, in_maps, core_ids=list(range(N_CORES)))
    _last_results = res
    out = np.empty((B_FULL, OUT_DIM), dtype=np.float32)
    for c in range(N_CORES):
        bg, og = divmod(c, OGRP)
        out[bg * BC:(bg + 1) * BC, og * OD:(og + 1) * OD] = (
            np.asarray(res.results[c]["out"]).astype(np.float32))
    return out
